# revision 22
# baseline (speedup 1.0000x reference)
"""CFBlock (GNN message passing) Trainium2 Bass kernel.

Sharding: edges sorted by dst; each of the 8 cores owns a contiguous range of
1250 destination nodes and all edges pointing into it. Each core:
  - (replicated) computes h_pre = LN1(x) @ W_pre + b_pre for ALL nodes and
    stores it as a bf16 table in DRAM,
  - gathers h_pre[src] for its edges with dma_gather, computes the edge filter
    GEMM, multiplies, and segment-sums via one-hot matmuls into PSUM windows
    of 128 destination nodes,
  - runs post-Linear + SiLU + residual + LN2 + FFN + residual for its nodes.
No collectives; the host concatenates the 8 output slices.

V2 notes (post-trace):
  - bulk DMAs (rbT, h_pre, out) ride HWDGE (nc.sync) so GpSimd is gather-only
  - one-hot built with 2-op tensor_scalar (subtract, is_equal) -- the 1-op
    is_equal encoding measured ~1us on HW
  - filter GEMM runs 2 blocks per PSUM bank; PSUM->SBUF casts ride nc.any
    (scheduler balances ACT/DVE); edge multiply is bf16 TT at 2x
  - gathers skip trailing pad rows (idx -1) with memset-zeroed tails
  - zero biases (the common case) skip their add instructions entirely;
    SiLU runs straight from PSUM on the scalar engine with fused bias
"""

import numpy as np
import ml_dtypes

import concourse.bass as bass
import concourse.mybir as mybir
from concourse import bacc
from concourse import library_config
from concourse.tile import TileContext
from concourse import bass_utils

BF16 = ml_dtypes.bfloat16

N_NODES = 10000
N_EDGES = 320000
D = 256          # d_model
DR = 128         # d_radial
DH = 256         # d_hidden
DFF = 1024
EPS = 1e-5
NCORES = 8
NPC = 1250       # nodes per core
NWIN = 10        # 128-node windows per core (last window: 98 valid nodes)
NPAD = 10112     # 79 * 128
NT = NPAD // 128  # 79 node tiles
XCH = 16         # node tiles per xT DMA chunk
RBCH = 16        # edge blocks per rbT DMA chunk
HB = 16          # h_pre tiles batched per DMA write

AF = mybir.ActivationFunctionType
OP = mybir.AluOpType

# packed f32 const columns: bpre, bpost, bff2, iota, bff1, eps, dstloc[, bfilt]
C_BPRE, C_BPOST, C_BFF2 = 0, 256, 512
C_IOTA, C_BFF1, C_EPS, C_DSTLOC = 768, 896, 904, 905
# packed bf16 const columns
W_PRE, W_FILT, W_POST, W_FF1, W_FF2, W_ID = 0, 512, 768, 1280, 3328, 5376
W_TOT = 5504


def _f32(a):
    return np.ascontiguousarray(a, dtype=np.float32)


def _bf(a):
    return np.ascontiguousarray(np.asarray(a, dtype=np.float32).astype(BF16))


def _build_program(Bw: int, flags: dict, phase: int = 4):
    nc = bacc.Bacc("TRN2", target_bir_lowering=False, debug=False)
    dt = mybir.dt

    has_bfilt = flags["bfilt"]
    has_bpre = flags["bpre"]
    has_bpost = flags["bpost"]
    has_bff2 = flags["bff2"]

    EPW = Bw * 128            # padded edges per window
    NBLK = NWIN * Bw          # padded blocks per core
    CW = C_DSTLOC + NBLK + (DH if has_bfilt else 0)
    C_BFILT = C_DSTLOC + NBLK

    # ---- I/O ----
    xnm_d = nc.dram_tensor("xnm", [NPAD, D], dt.bfloat16, kind="ExternalInput")
    xres_d = nc.dram_tensor("xres", [NWIN * 128, D], dt.float32, kind="ExternalInput")
    rbT_d = nc.dram_tensor("rbT", [DR, NBLK * 128], dt.bfloat16, kind="ExternalInput")
    ohp_d = nc.dram_tensor("ohp", [128, NBLK * 128], dt.bfloat16, kind="ExternalInput")
    gidx_d = nc.dram_tensor("gidx", [NWIN, 128, Bw * 8], dt.int16, kind="ExternalInput")
    cpack_d = nc.dram_tensor("cpack", [128, CW], dt.float32, kind="ExternalInput")
    wpack_d = nc.dram_tensor("wpack", [128, W_TOT], dt.bfloat16, kind="ExternalInput")
    out_d = nc.dram_tensor("out", [NWIN * 128, D], dt.float32, kind="ExternalOutput")

    with TileContext(nc) as tc:
        with (
            tc.tile_pool(name="consts", bufs=1) as consts,
            tc.tile_pool(name="dram", bufs=1, space="DRAM") as dramp,
            tc.tile_pool(name="n1", bufs=6) as n1p,
            tc.tile_pool(name="hout", bufs=2) as houtp,
            tc.tile_pool(name="rbt", bufs=3) as rbtp,
            tc.tile_pool(name="ohp", bufs=3) as ohpp,
            tc.tile_pool(name="gp", bufs=4) as gpp,
            tc.tile_pool(name="edge", bufs=8) as edgep,
            # PSUM budget is 8 banks of 2KB/partition, sized to exactly 8:
            # fps(2) + hagg/pps(2) + tr(2|1) + mm256(1) + f1ps(1) + gagg(0|1)
            tc.tile_pool(name="fps", bufs=2, space="PSUM") as fpsp,
            tc.tile_pool(name="hagg", bufs=2, space="PSUM") as haggp,
            tc.tile_pool(name="n2", bufs=2) as n2p,
            tc.tile_pool(name="trp", bufs=1 if has_bfilt else 2,
                         space="PSUM") as trp,
            tc.tile_pool(name="n2ps1", bufs=1, space="PSUM") as n2ps1,
            tc.tile_pool(name="n2ps2", bufs=1, space="PSUM") as n2ps2,
            tc.tile_pool(name="gaggp", bufs=1, space="PSUM") as gaggp,
        ):
            nc.gpsimd.load_library(library_config.mlp)
            cpk = consts.tile([128, CW], dt.float32, tag="cpack")
            nc.sync.dma_start(out=cpk[:], in_=cpack_d[:])
            wpk = consts.tile([128, W_TOT], dt.bfloat16, tag="wpack")
            nc.sync.dma_start(out=wpk[:], in_=wpack_d[:])
            gidx_sb = consts.tile([128, NWIN, Bw * 8], dt.int16, tag="gidx")
            nc.sync.dma_start(out=gidx_sb[:],
                              in_=gidx_d[:].rearrange("w p s -> p w s"))
            xnm_r = xnm_d[:].rearrange("(t p) n -> t p n", p=128)
            xnm_sb = consts.tile([128, NT, D], dt.bfloat16, tag="xbig")
            nc.sync.dma_start(out=xnm_sb[:],
                              in_=xnm_r.rearrange("t p n -> p t n"))
            xres_r = xres_d[:].rearrange("(w p) n -> w p n", p=128)
            xres_sb = consts.tile([128, NWIN, D], dt.float32, tag="xresb")
            nc.sync.dma_start(out=xres_sb[:],
                              in_=xres_r.rearrange("w p n -> p w n"))
            outb = consts.tile([128, NWIN, D], dt.float32, tag="outb")
            stds = consts.tile([128, NT + NWIN], dt.float32, tag="stds")
            mvall = consts.tile([128, NT, 2], dt.float32, tag="mvall")
            rstds = consts.tile([128, NT], dt.float32, tag="rstds")
            f1sil = consts.tile([128, NWIN, 8, 128], dt.bfloat16, tag="f1sil")

            bpre_sb = cpk[:, C_BPRE:C_BPRE + DH]
            bpost_sb = cpk[:, C_BPOST:C_BPOST + D]
            bff2_sb = cpk[:, C_BFF2:C_BFF2 + D]
            iota_sb = cpk[:, C_IOTA:C_IOTA + 128]
            bff1_sb = cpk[:, C_BFF1:C_BFF1 + 8]
            eps_sb = cpk[:, C_EPS:C_EPS + 1]
            dstloc_sb = cpk[:, C_DSTLOC:C_DSTLOC + NBLK]
            bfilt_sb = cpk[:, C_BFILT:C_BFILT + DH] if has_bfilt else None
            wpre_k = lambda k: wpk[:, W_PRE + k * DH:W_PRE + (k + 1) * DH]
            wfilt_sb = wpk[:, W_FILT:W_FILT + DH]
            wpost_k = lambda k: wpk[:, W_POST + k * D:W_POST + (k + 1) * D]
            wff1_k = lambda k: wpk[:, W_FF1 + k * DFF:W_FF1 + (k + 1) * DFF]
            wff2_s = lambda s: wpk[:, W_FF2 + s * D:W_FF2 + (s + 1) * D]
            ident_sb = wpk[:, W_ID:W_ID + 128]

            hpre_dram = dramp.tile([NPAD, D], dt.bfloat16, tag="hpre")
            hpre_r = hpre_dram[:].rearrange("(t p) n -> t p n", p=128)

            # ---- node phase 1: h_pre for all nodes ----
            # pass A: LN1 stats for all tiles, then one batched sqrt+recip
            for t in range(NT):
                stats = n1p.tile([128, 6], dt.float32, tag="bnst")
                nc.vector.bn_stats(out=stats[:], in_=xnm_sb[:, t, :])
                nc.vector.bn_aggr(out=mvall[:, t, :], in_=stats[:])
            nc.scalar.activation(stds[:, 0:NT], mvall[:, :, 1], AF.Sqrt,
                                 bias=eps_sb)
            nc.vector.reciprocal(out=rstds[:], in_=stds[:, 0:NT])
            # pass B: normalize + transpose + matmul
            h_big = None
            for t in range(NT):
                if t % HB == 0:
                    h_big = houtp.tile([128, HB, DH], dt.bfloat16, tag="hsb")
                x_sb = xnm_sb[:, t, :]
                z = n1p.tile([128, D], dt.bfloat16, tag="z")
                nc.vector.tensor_scalar(out=z[:], in0=x_sb,
                                        scalar1=mvall[:, t, 0:1],
                                        scalar2=rstds[:, t:t + 1],
                                        op0=OP.subtract, op1=OP.mult)
                ztps = trp.tile([128, 2, 128], dt.bfloat16, tag="tr")
                nc.tensor.transpose(ztps[:, 0, :], z[:, 0:128], ident_sb)
                nc.tensor.transpose(ztps[:, 1, :], z[:, 128:256], ident_sb)
                zT = n1p.tile([128, 2, 128], dt.bfloat16, tag="zT")
                nc.any.tensor_copy(out=zT[:, 0, :], in_=ztps[:, 0, :])
                nc.any.tensor_copy(out=zT[:, 1, :], in_=ztps[:, 1, :])
                pps = haggp.tile([128, DH], dt.float32, tag="hagg")
                nc.tensor.matmul(pps[:], lhsT=zT[:, 0, :],
                                 rhs=wpre_k(0), start=True, stop=False)
                nc.tensor.matmul(pps[:], lhsT=zT[:, 1, :],
                                 rhs=wpre_k(1), start=False, stop=True)
                if has_bpre:
                    nc.vector.tensor_tensor(out=h_big[:, t % HB, :], in0=pps[:],
                                            in1=bpre_sb, op=OP.add)
                else:
                    nc.any.tensor_copy(out=h_big[:, t % HB, :], in_=pps[:])
                if t % HB == HB - 1 or t == NT - 1:
                    t0 = (t // HB) * HB
                    nb = t - t0 + 1
                    nc.sync.dma_start(
                        out=hpre_r[t0:t0 + nb].rearrange("t p n -> p t n"),
                        in_=h_big[:, :nb, :])

            # ---- edge phase + per-window epilogue ----
            out_r = out_d[:].rearrange("(w p) n -> w p n", p=128)
            BH = Bw // 2  # blocks per half-window gather
            g_half = [None, None]
            rbt_chunks = {}
            oh_chunks = {}

            def get_rbt(j):
                k = j // RBCH
                if k not in rbt_chunks:
                    c0 = k * RBCH * 128
                    ncols = min(RBCH * 128, NBLK * 128 - c0)
                    t = rbtp.tile([128, RBCH * 128], dt.bfloat16, tag="rbt")
                    nc.sync.dma_start(out=t[:, :ncols],
                                      in_=rbT_d[:, c0:c0 + ncols])
                    rbt_chunks[k] = t
                return rbt_chunks[k]

            def get_oh(j):
                k = j // RBCH
                if k not in oh_chunks:
                    c0 = k * RBCH * 128
                    ncols = min(RBCH * 128, NBLK * 128 - c0)
                    t = ohpp.tile([128, RBCH * 128], dt.bfloat16, tag="ohp")
                    nc.sync.dma_start(out=t[:, :ncols],
                                      in_=ohp_d[:, c0:c0 + ncols])
                    oh_chunks[k] = t
                return oh_chunks[k]
            for w in range(NWIN if phase >= 2 else 0):
                for gh in range(2):
                    g_tile = gpp.tile([128, BH, DH], dt.bfloat16, tag="g")
                    g_half[gh] = g_tile
                    nc.gpsimd.dma_gather(
                        g_tile[:], hpre_dram[:],
                        gidx_sb[:, w, gh * BH * 8:(gh + 1) * BH * 8],
                        BH * 128, BH * 128, DH, single_packet=False)
                if phase < 3:
                    continue
                hagg = haggp.tile([128, DH], dt.float32, tag="hagg")
                gagg = None
                if has_bfilt:
                    gagg = gaggp.tile([128, DH], dt.float32, tag="gagg")
                for gh in range(2):
                    for p in range((BH + 1) // 2):
                        b0g = 2 * p
                        nb = min(2, BH - b0g)
                        j0 = w * Bw + gh * BH + b0g
                        # filter GEMM for up to two blocks into one PSUM bank
                        fps = fpsp.tile([128, 2, DH], dt.float32, tag="fps")
                        for i in range(nb):
                            rbt_sb = get_rbt(j0 + i)
                            boff = ((j0 + i) % RBCH) * 128
                            nc.tensor.matmul(fps[:, i, :],
                                             lhsT=rbt_sb[:, boff:boff + 128],
                                             rhs=wfilt_sb, start=True, stop=True)
                        fsb = edgep.tile([128, 2, DH], dt.bfloat16, tag="fsb")
                        nc.any.tensor_copy(out=fsb[:, :nb, :], in_=fps[:, :nb, :])
                        m_sb = edgep.tile([128, 2, DH], dt.bfloat16, tag="m")
                        nc.vector.tensor_tensor(
                            out=m_sb[:, :nb, :], in0=fsb[:, :nb, :],
                            in1=g_half[gh][:, b0g:b0g + nb, :], op=OP.mult)
                        for i in range(nb):
                            j = j0 + i
                            oh_sb = get_oh(j)
                            boff = (j % RBCH) * 128
                            oh = oh_sb[:, boff:boff + 128]
                            nc.tensor.matmul(hagg[:], lhsT=oh,
                                             rhs=m_sb[:, i, :],
                                             start=(j == w * Bw),
                                             stop=(j == w * Bw + Bw - 1))
                            if has_bfilt:
                                nc.tensor.matmul(gagg[:], lhsT=oh,
                                                 rhs=g_half[gh][:, b0g + i, :],
                                                 start=(j == w * Bw),
                                                 stop=(j == w * Bw + Bw - 1))

                if phase < 4:
                    continue
                # ---- epilogue for this window ----
                hagg_sb = n2p.tile([128, DH], dt.bfloat16, tag="haggsb")
                if has_bfilt:
                    tmpb = n2p.tile([128, DH], dt.float32, tag="tmpb")
                    nc.vector.tensor_tensor(out=tmpb[:], in0=gagg[:],
                                            in1=bfilt_sb, op=OP.mult)
                    nc.vector.tensor_tensor(out=hagg_sb[:], in0=hagg[:],
                                            in1=tmpb[:], op=OP.add)
                else:
                    nc.any.tensor_copy(out=hagg_sb[:], in_=hagg[:])
                tps = trp.tile([128, 2, 128], dt.bfloat16, tag="tr")
                nc.tensor.transpose(tps[:, 0, :], hagg_sb[:, 0:128], ident_sb)
                nc.tensor.transpose(tps[:, 1, :], hagg_sb[:, 128:256], ident_sb)
                haggT = n2p.tile([128, 2, 128], dt.bfloat16, tag="haggT")
                nc.any.tensor_copy(out=haggT[:, 0, :], in_=tps[:, 0, :])
                nc.any.tensor_copy(out=haggT[:, 1, :], in_=tps[:, 1, :])
                pops = n2ps1.tile([128, D], dt.float32, tag="mm256")
                nc.tensor.matmul(pops[:], lhsT=haggT[:, 0, :],
                                 rhs=wpost_k(0), start=True, stop=False)
                nc.tensor.matmul(pops[:], lhsT=haggT[:, 1, :],
                                 rhs=wpost_k(1), start=False, stop=True)
                if has_bpost:
                    ps_sb = n2p.tile([128, D], dt.float32, tag="pssb")
                    nc.vector.tensor_tensor(out=ps_sb[:], in0=pops[:],
                                            in1=bpost_sb, op=OP.add)
                    nc.scalar.activation(outb[:, w, :], ps_sb[:], AF.Silu)
                else:
                    nc.scalar.activation(outb[:, w, :], pops[:], AF.Silu)
                x1 = n2p.tile([128, D], dt.float32, tag="x1")
                nc.vector.tensor_tensor(out=x1[:], in0=outb[:, w, :],
                                        in1=xres_sb[:, w, :], op=OP.add)
                # LN2
                st2 = n1p.tile([128, 6], dt.float32, tag="bnst")
                nc.vector.bn_stats(out=st2[:], in_=x1[:])
                mv2 = n1p.tile([128, 2], dt.float32, tag="bnagg")
                nc.vector.bn_aggr(out=mv2[:], in_=st2[:])
                nc.scalar.activation(stds[:, NT + w:NT + w + 1], mv2[:, 1:2],
                                     AF.Sqrt, bias=eps_sb)
                rstd2 = n1p.tile([128, 1], dt.float32, tag="rstd")
                nc.vector.reciprocal(out=rstd2[:], in_=stds[:, NT + w:NT + w + 1])
                z2 = n2p.tile([128, D], dt.bfloat16, tag="z2")
                nc.vector.tensor_scalar(out=z2[:], in0=x1[:],
                                        scalar1=mv2[:, 0:1], scalar2=rstd2[:],
                                        op0=OP.subtract, op1=OP.mult)
                tps2 = trp.tile([128, 2, 128], dt.bfloat16, tag="tr")
                nc.tensor.transpose(tps2[:, 0, :], z2[:, 0:128], ident_sb)
                nc.tensor.transpose(tps2[:, 1, :], z2[:, 128:256], ident_sb)
                z2T = n2p.tile([128, 2, 128], dt.bfloat16, tag="z2T")
                nc.any.tensor_copy(out=z2T[:, 0, :], in_=tps2[:, 0, :])
                nc.any.tensor_copy(out=z2T[:, 1, :], in_=tps2[:, 1, :])
                for h in range(2):
                    f1ps = n2ps2.tile([128, 4, 128], dt.float32, tag="f1ps")
                    for s4 in range(4):
                        s = h * 4 + s4
                        nc.tensor.matmul(f1ps[:, s4, :],
                                         lhsT=wff1_k(0)[:, s * 128:(s + 1) * 128],
                                         rhs=z2T[:, 0, :], start=True, stop=False)
                        nc.tensor.matmul(f1ps[:, s4, :],
                                         lhsT=wff1_k(1)[:, s * 128:(s + 1) * 128],
                                         rhs=z2T[:, 1, :], start=False, stop=True)
                    for s4 in range(4):
                        s = h * 4 + s4
                        # silu(f1 + b_ff1) straight from PSUM; bias is
                        # per-partition in this [ff, node] layout
                        nc.scalar.activation(f1sil[:, w, s, :], f1ps[:, s4, :],
                                             AF.Silu, bias=bff1_sb[:, s:s + 1])
                f2ps = n2ps1.tile([128, D], dt.float32, tag="mm256")
                for s in range(8):
                    nc.tensor.matmul(f2ps[:], lhsT=f1sil[:, w, s, :],
                                     rhs=wff2_s(s),
                                     start=(s == 0), stop=(s == 7))
                if has_bff2:
                    o1 = n2p.tile([128, D], dt.float32, tag="o1")
                    nc.vector.tensor_tensor(out=o1[:], in0=f2ps[:], in1=bff2_sb,
                                            op=OP.add)
                    nc.vector.tensor_tensor(out=outb[:, w, :], in0=o1[:],
                                            in1=x1[:], op=OP.add)
                else:
                    nc.vector.tensor_tensor(out=outb[:, w, :], in0=f2ps[:],
                                            in1=x1[:], op=OP.add)
                nc.sync.dma_start(out=out_r[w], in_=outb[:, w, :])

    nc.compile()
    return nc


def _prep_inputs(x, radial_basis, src, dst, ln1_s, ln1_b, W_pre, b_pre,
                 W_filt, b_filt, W_post, b_post, ln2_s, ln2_b,
                 W_ff1, b_ff1, W_ff2, b_ff2):
    """Host-side staging: LN folds, edge sort/pad, per-core arrays."""
    x = _f32(x)
    rb = _f32(radial_basis)
    src = np.asarray(src).astype(np.int64)
    dst = np.asarray(dst).astype(np.int64)

    # fold LN1/LN2 scale+bias into the following Linear
    W_pre_f = _f32(ln1_s)[:, None] * _f32(W_pre)
    b_pre_f = _f32(ln1_b) @ _f32(W_pre) + _f32(b_pre)
    W_ff1_f = _f32(ln2_s)[:, None] * _f32(W_ff1)
    b_ff1_f = _f32(ln2_b) @ _f32(W_ff1) + _f32(b_ff1)

    order = np.argsort(dst, kind="stable")
    dst_s = dst[order]
    src_s = src[order]

    starts, tops = [], []
    for c in range(NCORES):
        for w in range(NWIN):
            starts.append(c * NPC + w * 128)
            tops.append(min(c * NPC + (w + 1) * 128, (c + 1) * NPC))
    edge_lo = np.searchsorted(dst_s, np.array(starts), side="left")
    edge_hi = np.searchsorted(dst_s, np.array(tops), side="left")
    counts = edge_hi - edge_lo
    Bw = max(2, int(np.max((counts + 127) // 128)))
    Bw += Bw % 2  # even, for half-window gathers
    EPW = Bw * 128
    NBLK = NWIN * Bw

    flags = dict(
        bfilt=bool(np.any(np.asarray(b_filt) != 0)),
        bpre=bool(np.any(b_pre_f != 0)),
        bpost=bool(np.any(np.asarray(b_post) != 0)),
        bff2=bool(np.any(np.asarray(b_ff2) != 0)),
    )

    wpre_bf = _bf(W_pre_f)
    cpack_common = np.concatenate([
        np.broadcast_to(b_pre_f, (128, DH)),
        np.broadcast_to(_f32(b_post), (128, D)),
        np.broadcast_to(_f32(b_ff2), (128, D)),
        np.broadcast_to(np.arange(128, dtype=np.float32), (128, 128)),
        np.ascontiguousarray(b_ff1_f.reshape(8, 128).T),
        np.full((128, 1), EPS, dtype=np.float32),
    ], axis=1).astype(np.float32)

    wpack = np.concatenate([
        wpre_bf[0:128], wpre_bf[128:256],
        _bf(W_filt),
        _bf(W_post)[0:128], _bf(W_post)[128:256],
        _bf(W_ff1_f)[0:128], _bf(W_ff1_f)[128:256],
        np.concatenate([_bf(W_ff2)[s * 128:(s + 1) * 128] for s in range(8)],
                       axis=1).reshape(128, 8 * D),
        _bf(np.eye(128, dtype=np.float32)),
    ], axis=1).astype(BF16)
    assert wpack.shape == (128, W_TOT), wpack.shape

    BH = Bw // 2
    per_core = []
    for c in range(NCORES):
        src_pad = np.zeros((NWIN, EPW), dtype=np.int64)
        dl_pad = np.full((NWIN, EPW), -1.0, dtype=np.float32)
        eids = np.full((NWIN, EPW), -1, dtype=np.int64)
        for w in range(NWIN):
            k = c * NWIN + w
            lo, hi = edge_lo[k], edge_hi[k]
            n = hi - lo
            src_pad[w, :n] = src_s[lo:hi]
            dl_pad[w, :n] = (dst_s[lo:hi] - (c * NPC + w * 128)).astype(np.float32)
            eids[w, :n] = order[lo:hi]

        flat_eids = eids.reshape(-1)
        rb_rows = np.zeros((NWIN * EPW, DR), dtype=np.float32)
        valid = flat_eids >= 0
        rb_rows[valid] = rb[flat_eids[valid]]
        rbT = np.ascontiguousarray(rb_rows.T).astype(BF16)

        gi = np.zeros((NWIN, 128, Bw * 8), dtype=np.int16)
        for w in range(NWIN):
            wrapped = src_pad[w].reshape(Bw * 8, 16).T.astype(np.int16)  # [16, S]
            gi[w] = np.tile(wrapped, (8, 1))

        dl = dl_pad.reshape(NBLK, 128).T.copy()  # [128, NBLK]

        # one-hot scatter matrices, streamed as [128, NBLK*128] bf16:
        # block j cols [j*128,(j+1)*128); row p = edge p of block j
        dflat = dl_pad.reshape(NBLK * 128).astype(np.int64)  # [j*128+p]
        ohp = np.zeros((128, NBLK * 128), dtype=BF16)
        eidx = np.nonzero(dflat >= 0)[0]
        jj = eidx // 128
        pp = eidx % 128
        ohp[pp, jj * 128 + dflat[eidx]] = 1.0

        xr = np.zeros((NWIN * 128, D), dtype=np.float32)
        xr[:NPC] = x[c * NPC:(c + 1) * NPC]

        parts = [cpack_common, dl]
        if flags["bfilt"]:
            parts.append(np.broadcast_to(_f32(b_filt), (128, DH)))
        cpack = _f32(np.concatenate(parts, axis=1))

        per_core.append(dict(rbT=rbT, gidx=gi, cpack=cpack, xres=xr, ohp=ohp))

    xpad = np.zeros((NPAD, D), dtype=np.float32)
    xpad[:N_NODES] = x
    consts = dict(xnm=_bf(xpad), wpack=wpack)
    return Bw, flags, consts, per_core


LAST_EXEC_TIME_NS = None
LAST_RESULTS = None


def kernel(**inputs) -> np.ndarray:
    global LAST_EXEC_TIME_NS, LAST_RESULTS
    Bw, flags, consts, per_core = _prep_inputs(**inputs)
    nc = _build_program(Bw, flags)
    in_maps = []
    for c in range(NCORES):
        m = dict(consts)
        m.update(per_core[c])
        in_maps.append(m)
    res = bass_utils.run_bass_kernel_spmd(nc, in_maps, list(range(NCORES)))
    LAST_EXEC_TIME_NS = getattr(res, "exec_time_ns", None)
    LAST_RESULTS = res
    out = np.concatenate(
        [res.results[c]["out"][:NPC] for c in range(NCORES)], axis=0
    )
    return np.ascontiguousarray(out, dtype=np.float32)
